# revision 1
# baseline (speedup 1.0000x reference)
"""Trainium2 Bass kernel for nn_KeplerDiffEq.

Computes, per orbit (4 orbits on 4 SBUF partitions):
  E = Kepler solve (Newton, seed E0 = M + e*sinM, 4 iterations)
  dr/ddr via the orbital-plane -> inertial rotation, out = [dr | ddr]  [4,6]

Design notes:
  - All trig via the ACT Sin table only (one table set, one ~2.7us load).
    cos(t) is obtained as -sin(t - pi/2); every Sin argument is kept inside
    [-pi, pi] where the spline is accurate (verified on HW: err ~6e-8 inside,
    blows up outside).
  - No ACT Sqrt (different table set + 65536-ULP budget): sqrt/rsqrt via the
    magic-constant seed + 3 Newton-Raphson steps on the vector engine.
  - ||r|| uses orthonormality of the rotation columns: ||r||^2 = x^2 + y^2.
  - The 2000-step damped reference loop stalls in f32 ~5e-6 from the true
    root; a converged Newton solution matches it to ~1e-5 relative.

Sharding: problem is tiny ("too small to shard") -> replicated SPMD on all
8 cores; core 0's output is returned.
"""
import sys

if "/opt/trn_rl_repo" not in sys.path:
    sys.path.insert(0, "/opt/trn_rl_repo")

import numpy as np

N_ORBITS = 4
N_IN = 28
N_OUT = 6
N_NEWTON = 4
HALF_PI = float(np.float32(np.pi / 2))
MU = 3.0
MAGIC = 0x5F3759DF

_cache = {}


def _build():
    import concourse.tile as tile
    from concourse import bacc, mybir

    AF = mybir.ActivationFunctionType
    ALU = mybir.AluOpType
    F32 = mybir.dt.float32
    I32 = mybir.dt.int32
    P = N_ORBITS

    nc = bacc.Bacc("TRN2", target_bir_lowering=False, debug=False)
    IN = nc.dram_tensor("inp", [P, N_IN], F32, kind="ExternalInput")
    OUT = nc.dram_tensor("out", [P, N_OUT], F32, kind="ExternalOutput")

    with tile.TileContext(nc) as tc:
        with tc.tile_pool(name="p", bufs=1) as pool:
            tin = pool.tile([P, N_IN], F32, tag="tin")
            nc.sync.dma_start(tin[:], IN.ap())

            e_ap = tin[:, 11:12]
            a_ap = tin[:, 10:11]
            mm_ap = tin[:, 12:13]
            m_ap = tin[:, 0:1]
            xy_ap = tin[:, 13:15]

            # ANG = base angles + {0,-pi/2} offsets; T = sin(ANG)
            # T cols: [sinM, s_w, n_w, n_W, s_W, s_i, s_W, n_W, n_i, spare]
            # (s_* = sin, n_* = -cos via sin(t - pi/2))
            ang = pool.tile([P, 10], F32, tag="ang")
            nc.vector.tensor_tensor(out=ang[:], in0=tin[:, 0:10],
                                    in1=tin[:, 16:26], op=ALU.add)
            T = pool.tile([P, 10], F32, tag="T")
            nc.scalar.activation(T[:], ang[:], AF.Sin)

            # ---- rotation-matrix build (off the Newton critical path) ----
            # A cols 0-5 = [n_w, -n_w, s_w, s_w, -s_w, -n_w]; cols 6-9 =
            # A2m = [s_w, s_w, -n_w, -n_w]
            A = pool.tile([P, 10], F32, tag="A")
            nc.vector.tensor_copy(A[:, 0:1], T[:, 2:3])
            nc.vector.tensor_scalar(out=A[:, 1:6:4],
                                    in0=T[:, 2:3].broadcast_to([P, 2]),
                                    scalar1=-1.0, scalar2=None, op0=ALU.mult)
            nc.vector.tensor_copy(A[:, 2:4], T[:, 1:2].broadcast_to([P, 2]))
            nc.vector.tensor_scalar(out=A[:, 4:5], in0=T[:, 1:2],
                                    scalar1=-1.0, scalar2=None, op0=ALU.mult)
            nc.vector.tensor_copy(A[:, 6:8], T[:, 1:2].broadcast_to([P, 2]))
            nc.vector.tensor_scalar(out=A[:, 8:10],
                                    in0=T[:, 2:3].broadcast_to([P, 2]),
                                    scalar1=-1.0, scalar2=None, op0=ALU.mult)

            # C = [c11,c21,c31,c12,c22,c32]: first factors A6 * [n_W,s_W,s_i]x2
            C = pool.tile([P, 6], F32, tag="C")
            B_b = T[:, 3:6].unsqueeze(1).broadcast_to([P, 2, 3])
            nc.vector.tensor_tensor(
                out=C[:].rearrange("p (h j) -> p h j", h=2),
                in0=A[:, 0:6].rearrange("p (h j) -> p h j", h=2),
                in1=B_b, op=ALU.mult)
            # second terms on cols (0,1,3,4):
            p2 = pool.tile([P, 2], F32, tag="p2")
            nc.vector.tensor_scalar(out=p2[:], in0=T[:, 6:8],
                                    scalar1=T[:, 8:9], scalar2=None,
                                    op0=ALU.mult)
            Gm = pool.tile([P, 4], F32, tag="Gm")
            nc.vector.tensor_tensor(
                out=Gm[:].rearrange("p (h j) -> p h j", h=2),
                in0=A[:, 6:10].rearrange("p (h j) -> p h j", h=2),
                in1=p2[:].unsqueeze(1).broadcast_to([P, 2, 2]), op=ALU.mult)
            C2 = pool.tile([P, 6], F32, tag="C2")
            main4_in = C[:].rearrange("p (h j) -> p h j", h=2)[:, :, 0:2]
            main4_out = C2[:].rearrange("p (h j) -> p h j", h=2)[:, :, 0:2]
            nc.vector.tensor_tensor(out=main4_out, in0=main4_in,
                                    in1=Gm[:].rearrange("p (h j) -> p h j", h=2),
                                    op=ALU.add)
            nc.vector.tensor_copy(C2[:, 2:6:3], C[:, 2:6:3])
            C = C2

            # ---- V = [MU*a, MU*a*(1-e^2), x^2+y^2]; Y = rsqrt(V) ----
            V = pool.tile([P, 3], F32, tag="V")
            nc.vector.tensor_scalar(out=V[:, 0:1], in0=a_ap, scalar1=MU,
                                    scalar2=None, op0=ALU.mult)
            e2t = pool.tile([P, 1], F32, tag="e2t")
            nc.vector.tensor_scalar(out=e2t[:], in0=tin[:, 11:12],
                                    scalar1=e_ap, scalar2=None, op0=ALU.mult)
            ome2 = pool.tile([P, 1], F32, tag="ome2")
            nc.vector.tensor_tensor(out=ome2[:], in0=tin[:, 15:16],
                                    in1=e2t[:], op=ALU.subtract)
            nc.vector.tensor_tensor(out=V[:, 1:2], in0=V[:, 0:1],
                                    in1=ome2[:], op=ALU.mult)
            sqxy = pool.tile([P, 2], F32, tag="sqxy")
            nc.vector.tensor_tensor(out=sqxy[:], in0=xy_ap, in1=xy_ap,
                                    op=ALU.mult)
            nc.vector.tensor_tensor(out=V[:, 2:3], in0=sqxy[:, 0:1],
                                    in1=sqxy[:, 1:2], op=ALU.add)

            Y = pool.tile([P, 3], F32, tag="Y")
            sh = pool.tile([P, 3], I32, tag="sh")
            nc.vector.tensor_scalar(out=sh[:], in0=V[:].bitcast(I32),
                                    scalar1=1, scalar2=None,
                                    op0=ALU.logical_shift_right)
            nc.vector.tensor_scalar(out=Y[:].bitcast(I32), in0=sh[:],
                                    scalar1=MAGIC, scalar2=-1,
                                    op0=ALU.subtract, op1=ALU.mult)
            for it in range(3):
                t_a = pool.tile([P, 3], F32, tag=f"nra{it}")
                nc.vector.tensor_tensor(out=t_a[:], in0=Y[:], in1=Y[:],
                                        op=ALU.mult)
                nc.vector.tensor_tensor(out=t_a[:], in0=t_a[:], in1=V[:],
                                        op=ALU.mult)
                nc.vector.tensor_scalar(out=t_a[:], in0=t_a[:],
                                        scalar1=-0.5, scalar2=1.5,
                                        op0=ALU.mult, op1=ALU.add)
                Y2 = pool.tile([P, 3], F32, tag=f"nry{it}")
                nc.vector.tensor_tensor(out=Y2[:], in0=Y[:], in1=t_a[:],
                                        op=ALU.mult)
                Y = Y2
            SQ = pool.tile([P, 2], F32, tag="SQ")
            nc.vector.tensor_tensor(out=SQ[:], in0=V[:, 0:2], in1=Y[:, 0:2],
                                    op=ALU.mult)
            t1k = pool.tile([P, 1], F32, tag="t1k")
            nc.vector.tensor_scalar(out=t1k[:], in0=mm_ap, scalar1=mm_ap,
                                    scalar2=a_ap, op0=ALU.mult, op1=ALU.mult)
            t2k = pool.tile([P, 1], F32, tag="t2k")
            nc.vector.tensor_scalar(out=t2k[:], in0=t1k[:], scalar1=a_ap,
                                    scalar2=a_ap, op0=ALU.mult, op1=ALU.mult)

            # ---- Newton-Kepler: EE = [F, F - pi/2], F = E - M ----
            P0 = pool.tile([P, 1], F32, tag="P0")
            nc.vector.tensor_scalar(out=P0[:], in0=T[:, 0:1], scalar1=e_ap,
                                    scalar2=None, op0=ALU.mult)
            EE = pool.tile([P, 2], F32, tag="EE0")
            nc.vector.tensor_tensor(out=EE[:], in0=P0[:].broadcast_to([P, 2]),
                                    in1=tin[:, 16:19:2], op=ALU.add)

            S = None
            for it in range(N_NEWTON):
                S = pool.tile([P, 2], F32, tag=f"S{it}")
                nc.scalar.activation(S[:], EE[:], AF.Sin, bias=m_ap)
                nnum = pool.tile([P, 1], F32, tag=f"nn{it}")
                nc.vector.tensor_scalar(out=nnum[:], in0=S[:, 0:1],
                                        scalar1=e_ap, scalar2=EE[:, 0:1],
                                        op0=ALU.mult, op1=ALU.subtract)
                den = pool.tile([P, 1], F32, tag=f"dn{it}")
                nc.vector.tensor_scalar(out=den[:], in0=S[:, 1:2],
                                        scalar1=e_ap, scalar2=1.0,
                                        op0=ALU.mult, op1=ALU.add)
                rec = pool.tile([P, 1], F32, tag=f"rc{it}")
                nc.vector.reciprocal(rec[:], den[:])
                dF = pool.tile([P, 1], F32, tag=f"dF{it}")
                nc.vector.tensor_tensor(out=dF[:], in0=nnum[:], in1=rec[:],
                                        op=ALU.mult)
                EE2 = pool.tile([P, 2], F32, tag=f"EE{it + 1}")
                nc.vector.tensor_tensor(out=EE2[:], in0=EE[:],
                                        in1=dF[:].broadcast_to([P, 2]),
                                        op=ALU.add)
                EE = EE2

            # final trig at converged E
            S5 = pool.tile([P, 2], F32, tag="S5")
            nc.scalar.activation(S5[:], EE[:], AF.Sin, bias=m_ap)
            den5 = pool.tile([P, 1], F32, tag="den5")
            nc.vector.tensor_scalar(out=den5[:], in0=S5[:, 1:2],
                                    scalar1=e_ap, scalar2=1.0,
                                    op0=ALU.mult, op1=ALU.add)

            # ---- tail ----
            rcen = pool.tile([P, 1], F32, tag="rcen")
            nc.vector.tensor_scalar(out=rcen[:], in0=den5[:], scalar1=a_ap,
                                    scalar2=None, op0=ALU.mult)
            rcinv = pool.tile([P, 1], F32, tag="rcinv")
            nc.vector.reciprocal(rcinv[:], rcen[:])

            sc2 = pool.tile([P, 2], F32, tag="sc2")
            nc.vector.tensor_scalar(out=sc2[:], in0=SQ[:], scalar1=rcinv[:],
                                    scalar2=None, op0=ALU.mult)
            ds2 = pool.tile([P, 2], F32, tag="ds2")
            nc.vector.tensor_tensor(out=ds2[:], in0=sc2[:], in1=S5[:],
                                    op=ALU.mult)
            PQ = pool.tile([P, 4], F32, tag="PQ")
            nc.vector.tensor_scalar(out=PQ[:, 0:4:2], in0=ds2[:],
                                    scalar1=-1.0, scalar2=None, op0=ALU.mult)
            t3k = pool.tile([P, 1], F32, tag="t3k")
            nc.vector.tensor_scalar(out=t3k[:], in0=t2k[:], scalar1=rcinv[:],
                                    scalar2=rcinv[:], op0=ALU.mult,
                                    op1=ALU.mult)
            kk = pool.tile([P, 1], F32, tag="kk")
            nc.vector.tensor_scalar(out=kk[:], in0=t3k[:], scalar1=Y[:, 2:3],
                                    scalar2=-1.0, op0=ALU.mult, op1=ALU.mult)
            nc.vector.tensor_tensor(out=PQ[:, 1:4:2],
                                    in0=kk[:].broadcast_to([P, 2]),
                                    in1=xy_ap, op=ALU.mult)

            O1 = pool.tile([P, 6], F32, tag="O1")
            nc.vector.tensor_tensor(
                out=O1[:].rearrange("p (h j) -> p h j", h=2),
                in0=C[:, 0:3].unsqueeze(1).broadcast_to([P, 2, 3]),
                in1=PQ[:, 0:2].unsqueeze(2).broadcast_to([P, 2, 3]),
                op=ALU.mult)
            O2 = pool.tile([P, 6], F32, tag="O2")
            nc.vector.tensor_tensor(
                out=O2[:].rearrange("p (h j) -> p h j", h=2),
                in0=C[:, 3:6].unsqueeze(1).broadcast_to([P, 2, 3]),
                in1=PQ[:, 2:4].unsqueeze(2).broadcast_to([P, 2, 3]),
                op=ALU.mult)
            Ot = pool.tile([P, 6], F32, tag="Ot")
            nc.vector.tensor_tensor(out=Ot[:], in0=O1[:], in1=O2[:],
                                    op=ALU.add)
            nc.sync.dma_start(OUT.ap(), Ot[:])

    nc.compile()
    return nc


def _pack(a, e, i, omega, Omega, mean_motion, mean_anomaly, x):
    P = N_ORBITS
    IN = np.zeros((P, N_IN), np.float32)
    M = np.full((P,), np.float32(mean_anomaly), np.float32)
    w = np.asarray(omega, np.float32).reshape(P)
    W = np.asarray(Omega, np.float32).reshape(P)
    ii = np.asarray(i, np.float32).reshape(P)
    IN[:, 0] = M
    IN[:, 1] = w
    IN[:, 2] = w
    IN[:, 3] = W
    IN[:, 4] = W
    IN[:, 5] = ii
    IN[:, 6] = W
    IN[:, 7] = W
    IN[:, 8] = ii
    IN[:, 9] = M
    IN[:, 10] = np.asarray(a, np.float32).reshape(P)
    IN[:, 11] = np.asarray(e, np.float32).reshape(P)
    IN[:, 12] = np.asarray(mean_motion, np.float32).reshape(P)
    IN[:, 13] = np.asarray(x, np.float32)[:, 0]
    IN[:, 14] = np.asarray(x, np.float32)[:, 1]
    IN[:, 15] = 1.0
    IN[:, 16:26] = np.array([0, 0, -HALF_PI, -HALF_PI, 0,
                             0, 0, -HALF_PI, -HALF_PI, 0], np.float32)
    return IN


def kernel(a, e, i, omega, Omega, mean_motion, mean_anomaly, x, _trace=False):
    from concourse.bass_utils import run_bass_kernel_spmd

    if "nc" not in _cache:
        _cache["nc"] = _build()
    nc = _cache["nc"]

    IN = _pack(a, e, i, omega, Omega, mean_motion, mean_anomaly, x)
    n_cores = 1 if _trace else 8
    res = run_bass_kernel_spmd(nc, [{"inp": IN}] * n_cores,
                               core_ids=list(range(n_cores)), trace=_trace)
    out = res.results[0]["out"].astype(np.float32)
    if _trace:
        _cache["last_result"] = res
    return out



# revision 16
# speedup vs baseline: 1.1730x; 1.1730x over previous
"""Trainium2 Bass kernel for nn_KeplerDiffEq.

Computes, per orbit (4 orbits on 4 SBUF partitions):
  E = Kepler solve (Newton, seed E0 = M + e*sinM, 2 iterations, final
  trig via first-order rotation sin(E1-d) ~ sinE1 - d*cosE1)
  dr/ddr via the orbital-plane -> inertial rotation, out = [dr | ddr]  [4,6]

Accuracy (vs the 2000-step damped-Newton f32 reference, worst case over
M in [0,1)): Newton-2+rotate ~6e-4, magic-rsqrt+1NR ~3.5e-3 -> ~4e-3
total, vs the 2e-2 gate.

Schedule: the serial Newton chain (3 Sin ACTs + ~21 tiny vector ops)
runs on Scalar+Vector; everything independent of E (rotation matrix C,
V = [3a, 3a(1-e^2), x^2+y^2], magic-rsqrt refine, u2 = -mm^2 a^3
rsqrt(xx+yy)*[x,y]) runs concurrently on GpSimd (tensor_tensor only --
Pool rejects TensorScalarPtr and 32-bit shifts, so constants arrive as
input lanes and the 2-op magic seed runs in a Vector ACT-wait gap).
The mm^2 a^3 product chain runs on the otherwise-idle Scalar engine via
Square/Copy activations. Host packs angles with -pi/2 offsets pre-added
so cos comes from the single Sin table (no Sqrt table).

Sharding: problem is tiny ("too small to shard") -> replicated SPMD on
all 8 cores; core 0's output is returned.
"""
import sys

if "/opt/trn_rl_repo" not in sys.path:
    sys.path.insert(0, "/opt/trn_rl_repo")

import numpy as np

N_ORBITS = 4
N_IN = 25
N_OUT = 6
HALF_PI = float(np.float32(np.pi / 2))
MAGIC = 0x5F3759DF

_cache = {}


def _build():
    import concourse.tile as tile
    from concourse import bacc, mybir

    AF = mybir.ActivationFunctionType
    ALU = mybir.AluOpType
    F32 = mybir.dt.float32
    I32 = mybir.dt.int32
    P = N_ORBITS

    nc = bacc.Bacc("TRN2", target_bir_lowering=False, debug=False)
    IN = nc.dram_tensor("inp", [P, N_IN], F32, kind="ExternalInput")
    OUT = nc.dram_tensor("out", [P, N_OUT], F32, kind="ExternalOutput")

    with tile.TileContext(nc) as tc:
        with tc.tile_pool(name="p", bufs=1) as pool:
            tin = pool.tile([P, N_IN], F32, tag="tin")
            nc.sync.dma_start(tin[:], IN.ap())

            m_ap = tin[:, 0:1]
            e_ap = tin[:, 11:12]
            a_ap = tin[:, 12:13]
            mm_ap = tin[:, 13:14]
            xy_ap = tin[:, 14:16]
            offs2 = tin[:, 16:18]    # [0, -pi/2]
            sgn_pm = tin[:, 18:20]   # [-1, +1]
            neg1 = tin[:, 18:19]
            c3 = tin[:, 20:21]       # 3.0
            nhalf = tin[:, 21:22]    # -0.5
            c1p5 = tin[:, 22:23]     # 1.5
            nxy = tin[:, 23:25]      # [-x, -y]

            # T = sin(angles): cols 0..10 =
            # [M, w-pi/2, w, W, W-pi/2, w, pi/2-w, i, i-pi/2, W-pi/2, W]
            # -> [sM, n_w, s_w, s_W, n_W, s_w, c_w, s_i, n_i, n_W, s_W]
            # (s_* = sin, n_* = -cos, c_* = +cos)
            T = pool.tile([P, 11], F32, tag="T")
            nc.scalar.activation(T[:], tin[:, 0:11], AF.Sin)
            sa1 = pool.tile([P, 1], F32, tag="sa1")  # mm^2
            nc.scalar.square(sa1[:], mm_ap)

            # ---- GpSimd side-channel (everything independent of E) ----
            # V = [3a, 3a(1-e^2), x^2+y^2]
            V = pool.tile([P, 3], F32, tag="V")
            nc.gpsimd.tensor_tensor(out=V[:, 0:1], in0=a_ap, in1=c3,
                                    op=ALU.mult)
            e2g = pool.tile([P, 1], F32, tag="e2g")
            nc.gpsimd.tensor_tensor(out=e2g[:], in0=e_ap, in1=e_ap,
                                    op=ALU.mult)
            tv = pool.tile([P, 1], F32, tag="tv")   # e^2 * 3a
            nc.gpsimd.tensor_tensor(out=tv[:], in0=e2g[:], in1=V[:, 0:1],
                                    op=ALU.mult)
            nc.gpsimd.tensor_tensor(out=V[:, 1:2], in0=V[:, 0:1], in1=tv[:],
                                    op=ALU.subtract)
            ne = pool.tile([P, 1], F32, tag="ne")   # -e
            nc.gpsimd.tensor_tensor(out=ne[:], in0=e_ap, in1=neg1,
                                    op=ALU.mult)
            nae = pool.tile([P, 1], F32, tag="nae")  # -a*e
            nc.gpsimd.tensor_tensor(out=nae[:], in0=ne[:], in1=a_ap,
                                    op=ALU.mult)
            # rotation-matrix pieces that only need T
            C = pool.tile([P, 6], F32, tag="C")  # [c11,c21,c31,c12,c22,c32]
            Cv = C[:].rearrange("p (h j) -> p h j", h=2)[:, :, 0:2]
            nc.gpsimd.tensor_tensor(out=C[:, 2:3], in0=T[:, 2:3],
                                    in1=T[:, 7:8], op=ALU.mult)  # c31=sw*si
            nw2 = pool.tile([P, 2], F32, tag="nw2")  # [cw, -sw]
            nc.gpsimd.tensor_tensor(out=nw2[:], in0=T[:, 1:3],
                                    in1=neg1.broadcast_to([P, 2]),
                                    op=ALU.mult)
            LWc = pool.tile([P, 2], F32, tag="LWc")  # [cW, sW]
            nc.gpsimd.tensor_tensor(out=LWc[:], in0=T[:, 9:11], in1=sgn_pm,
                                    op=ALU.mult)
            nc.gpsimd.tensor_tensor(out=C[:, 5:6], in0=nw2[:, 0:1],
                                    in1=T[:, 7:8], op=ALU.mult)  # c32=cw*si
            # (magic-rsqrt refine emitted after the Vector seed ops below --
            # tile deps are tracked in emission order, so a pool read of Y
            # emitted before Y's write would silently get no dependency)
            Y = pool.tile([P, 3], F32, tag="Y")

            # ---- Vector: idle-start fillers, then the Newton chain ----
            sq2 = pool.tile([P, 2], F32, tag="sq2")
            nc.vector.tensor_tensor(out=sq2[:], in0=xy_ap, in1=xy_ap,
                                    op=ALU.mult)
            nc.vector.tensor_tensor(out=V[:, 2:3], in0=sq2[:, 0:1],
                                    in1=sq2[:, 1:2], op=ALU.add)

            # seed: F0 = e*sinM; EE0 = [F0, F0 - pi/2]
            F0 = pool.tile([P, 1], F32, tag="F0")
            nc.vector.tensor_scalar(out=F0[:], in0=T[:, 0:1], scalar1=e_ap,
                                    scalar2=None, op0=ALU.mult)
            EE0 = pool.tile([P, 2], F32, tag="EE0")
            nc.vector.tensor_tensor(out=EE0[:], in0=F0[:].broadcast_to([P, 2]),
                                    in1=offs2, op=ALU.add)
            S1 = pool.tile([P, 2], F32, tag="S1")
            nc.scalar.activation(S1[:], EE0[:], AF.Sin, bias=m_ap)
            sa3 = pool.tile([P, 1], F32, tag="sa3")  # mm^2 a
            nc.scalar.mul(sa3[:], sa1[:], a_ap)
            sa2 = pool.tile([P, 1], F32, tag="sa2")  # a^2
            nc.scalar.square(sa2[:], a_ap)

            # gap fillers during S1: magic-rsqrt seed (shift needs Vector)
            sh = pool.tile([P, 3], I32, tag="sh")
            nc.vector.tensor_scalar(out=sh[:], in0=V[:].bitcast(I32),
                                    scalar1=1, scalar2=None,
                                    op0=ALU.logical_shift_right)
            nc.vector.tensor_scalar(out=Y[:].bitcast(I32), in0=sh[:],
                                    scalar1=MAGIC, scalar2=-1,
                                    op0=ALU.subtract, op1=ALU.mult)

            # GpSimd: magic-rsqrt Newton-Raphson refine + SQpm
            nr = pool.tile([P, 3], F32, tag="nr")
            nc.gpsimd.tensor_tensor(out=nr[:], in0=Y[:], in1=Y[:],
                                    op=ALU.mult)
            nc.gpsimd.tensor_tensor(out=nr[:], in0=nr[:], in1=V[:],
                                    op=ALU.mult)
            nc.gpsimd.tensor_tensor(out=nr[:], in0=nr[:],
                                    in1=nhalf.broadcast_to([P, 3]),
                                    op=ALU.mult)
            nc.gpsimd.tensor_tensor(out=nr[:], in0=nr[:],
                                    in1=c1p5.broadcast_to([P, 3]),
                                    op=ALU.add)
            Y1 = pool.tile([P, 3], F32, tag="Y1")
            nc.gpsimd.tensor_tensor(out=Y1[:], in0=Y[:], in1=nr[:],
                                    op=ALU.mult)
            # SQpm = [-sqrt(3a), +sqrt(3a(1-e^2))]
            SQ = pool.tile([P, 2], F32, tag="SQ")
            nc.gpsimd.tensor_tensor(out=SQ[:], in0=V[:, 0:2], in1=Y1[:, 0:2],
                                    op=ALU.mult)
            SQpm = pool.tile([P, 2], F32, tag="SQpm")
            nc.gpsimd.tensor_tensor(out=SQpm[:], in0=SQ[:], in1=sgn_pm,
                                    op=ALU.mult)

            # Newton iteration 1
            f1 = pool.tile([P, 1], F32, tag="f1")
            nc.vector.tensor_scalar(out=f1[:], in0=S1[:, 0:1], scalar1=ne[:],
                                    scalar2=F0[:], op0=ALU.mult, op1=ALU.add)
            d1 = pool.tile([P, 1], F32, tag="d1")
            nc.vector.tensor_scalar(out=d1[:], in0=S1[:, 1:2], scalar1=e_ap,
                                    scalar2=1.0, op0=ALU.mult, op1=ALU.add)
            r1 = pool.tile([P, 1], F32, tag="r1")
            nc.vector.reciprocal(r1[:], d1[:])
            dF1 = pool.tile([P, 1], F32, tag="dF1")
            nc.vector.tensor_tensor(out=dF1[:], in0=f1[:], in1=r1[:],
                                    op=ALU.mult)
            EE1 = pool.tile([P, 2], F32, tag="EE1")
            nc.vector.tensor_tensor(out=EE1[:], in0=EE0[:],
                                    in1=dF1[:].broadcast_to([P, 2]),
                                    op=ALU.subtract)
            S2 = pool.tile([P, 2], F32, tag="S2")
            nc.scalar.activation(S2[:], EE1[:], AF.Sin, bias=m_ap)
            sa4 = pool.tile([P, 1], F32, tag="sa4")  # mm^2 a^3
            nc.scalar.mul(sa4[:], sa3[:], sa2[:])
            sa5 = pool.tile([P, 1], F32, tag="sa5")  # mm^2 a^3 rsqrt(xx+yy)
            nc.scalar.mul(sa5[:], sa4[:], Y1[:, 2:3])
            # u2 = -mm^2 a^3 rsqrt(xx+yy) * [x, y]  (sign via -x,-y lanes)
            u2 = pool.tile([P, 2], F32, tag="u2")
            nc.gpsimd.tensor_tensor(out=u2[:], in0=nxy,
                                    in1=sa5[:].broadcast_to([P, 2]),
                                    op=ALU.mult)

            # gap fillers during S2: rotation-matrix outer products
            C4m = pool.tile([P, 2, 2], F32, tag="C4m")
            nc.vector.tensor_tensor(
                out=C4m[:], in0=nw2[:].unsqueeze(2).broadcast_to([P, 2, 2]),
                in1=LWc[:].unsqueeze(1).broadcast_to([P, 2, 2]), op=ALU.mult)
            z2 = pool.tile([P, 2], F32, tag="z2")  # [-sw*ci, -cw*ci]
            nc.vector.tensor_scalar(out=z2[:], in0=T[:, 5:7],
                                    scalar1=T[:, 8:9], scalar2=None,
                                    op0=ALU.mult)

            # GpSimd finishes C once z2/C4m land
            Cb4 = pool.tile([P, 2, 2], F32, tag="Cb4")
            nc.gpsimd.tensor_tensor(
                out=Cb4[:], in0=z2[:].unsqueeze(2).broadcast_to([P, 2, 2]),
                in1=T[:, 3:5].unsqueeze(1).broadcast_to([P, 2, 2]),
                op=ALU.mult)
            nc.gpsimd.tensor_tensor(out=Cv, in0=C4m[:], in1=Cb4[:],
                                    op=ALU.add)

            # Newton iteration 2 + first-order trig rotation
            f2 = pool.tile([P, 1], F32, tag="f2")
            nc.vector.tensor_scalar(out=f2[:], in0=S2[:, 0:1], scalar1=ne[:],
                                    scalar2=EE1[:, 0:1], op0=ALU.mult,
                                    op1=ALU.add)
            d2 = pool.tile([P, 1], F32, tag="d2")
            nc.vector.tensor_scalar(out=d2[:], in0=S2[:, 1:2], scalar1=e_ap,
                                    scalar2=1.0, op0=ALU.mult, op1=ALU.add)
            r2 = pool.tile([P, 1], F32, tag="r2")
            nc.vector.reciprocal(r2[:], d2[:])
            dF2 = pool.tile([P, 1], F32, tag="dF2")
            nc.vector.tensor_tensor(out=dF2[:], in0=f2[:], in1=r2[:],
                                    op=ALU.mult)
            # S5 = [sin(Ef), cos(Ef)], Ef = E1 - dF2:
            #   sin ~ S2_0 + dF2*S2_1 ; cos ~ dF2*S2_0 - S2_1
            S5 = pool.tile([P, 2], F32, tag="S5")
            nc.vector.tensor_scalar(out=S5[:, 0:1], in0=S2[:, 1:2],
                                    scalar1=dF2[:], scalar2=S2[:, 0:1],
                                    op0=ALU.mult, op1=ALU.add)
            nc.vector.tensor_scalar(out=S5[:, 1:2], in0=S2[:, 0:1],
                                    scalar1=dF2[:], scalar2=S2[:, 1:2],
                                    op0=ALU.mult, op1=ALU.subtract)

            # ---- tail ----
            rcen = pool.tile([P, 1], F32, tag="rcen")  # a(1 - e cosEf)
            nc.vector.tensor_scalar(out=rcen[:], in0=S5[:, 1:2],
                                    scalar1=nae[:], scalar2=a_ap,
                                    op0=ALU.mult, op1=ALU.add)
            rci = pool.tile([P, 1], F32, tag="rci")
            nc.vector.reciprocal(rci[:], rcen[:])
            sc2 = pool.tile([P, 2], F32, tag="sc2")
            nc.vector.tensor_scalar(out=sc2[:], in0=SQpm[:], scalar1=rci[:],
                                    scalar2=None, op0=ALU.mult)
            PQ = pool.tile([P, 4], F32, tag="PQ")  # [dx, px, dy, qx]
            nc.vector.tensor_tensor(out=PQ[:, 0:4:2], in0=sc2[:], in1=S5[:],
                                    op=ALU.mult)
            nc.vector.tensor_scalar(out=PQ[:, 1:4:2], in0=u2[:],
                                    scalar1=rci[:], scalar2=rci[:],
                                    op0=ALU.mult, op1=ALU.mult)

            O1 = pool.tile([P, 6], F32, tag="O1")
            nc.vector.tensor_tensor(
                out=O1[:].rearrange("p (h j) -> p h j", h=2),
                in0=C[:, 0:3].unsqueeze(1).broadcast_to([P, 2, 3]),
                in1=PQ[:, 0:2].unsqueeze(2).broadcast_to([P, 2, 3]),
                op=ALU.mult)
            O2 = pool.tile([P, 6], F32, tag="O2")
            nc.vector.tensor_tensor(
                out=O2[:].rearrange("p (h j) -> p h j", h=2),
                in0=C[:, 3:6].unsqueeze(1).broadcast_to([P, 2, 3]),
                in1=PQ[:, 2:4].unsqueeze(2).broadcast_to([P, 2, 3]),
                op=ALU.mult)
            Ot = pool.tile([P, 6], F32, tag="Ot")
            nc.vector.tensor_tensor(out=Ot[:], in0=O1[:], in1=O2[:],
                                    op=ALU.add)
            nc.sync.dma_start(OUT.ap(), Ot[:])

    nc.compile()
    return nc


def _pack(a, e, i, omega, Omega, mean_motion, mean_anomaly, x):
    P = N_ORBITS
    IN = np.zeros((P, N_IN), np.float32)
    M = np.full((P,), np.float32(mean_anomaly), np.float32)
    w = np.asarray(omega, np.float32).reshape(P)
    W = np.asarray(Omega, np.float32).reshape(P)
    ii = np.asarray(i, np.float32).reshape(P)
    xf = np.asarray(x, np.float32)
    IN[:, 0] = M
    IN[:, 1] = w - HALF_PI
    IN[:, 2] = w
    IN[:, 3] = W
    IN[:, 4] = W - HALF_PI
    IN[:, 5] = w
    IN[:, 6] = HALF_PI - w
    IN[:, 7] = ii
    IN[:, 8] = ii - HALF_PI
    IN[:, 9] = W - HALF_PI
    IN[:, 10] = W
    IN[:, 11] = np.asarray(e, np.float32).reshape(P)
    IN[:, 12] = np.asarray(a, np.float32).reshape(P)
    IN[:, 13] = np.asarray(mean_motion, np.float32).reshape(P)
    IN[:, 14] = xf[:, 0]
    IN[:, 15] = xf[:, 1]
    IN[:, 16] = 0.0
    IN[:, 17] = -HALF_PI
    IN[:, 18] = -1.0
    IN[:, 19] = 1.0
    IN[:, 20] = 3.0
    IN[:, 21] = -0.5
    IN[:, 22] = 1.5
    IN[:, 23] = -xf[:, 0]
    IN[:, 24] = -xf[:, 1]
    return IN


def kernel(a, e, i, omega, Omega, mean_motion, mean_anomaly, x, _trace=False):
    from concourse.bass_utils import run_bass_kernel_spmd

    if "nc" not in _cache:
        _cache["nc"] = _build()
    nc = _cache["nc"]

    IN = _pack(a, e, i, omega, Omega, mean_motion, mean_anomaly, x)
    n_cores = 1 if _trace else 8
    res = run_bass_kernel_spmd(nc, [{"inp": IN}] * n_cores,
                               core_ids=list(range(n_cores)), trace=_trace)
    out = res.results[0]["out"].astype(np.float32)
    if _trace:
        _cache["last_result"] = res
    return out


# revision 17
# speedup vs baseline: 1.3016x; 1.1096x over previous
"""Trainium2 Bass kernel for nn_KeplerDiffEq.

Computes, per orbit (4 orbits on 4 SBUF partitions):
  E = Kepler solve (Newton, seed E0 = M + e*sinM, 2 iterations, final
  trig via first-order rotation sin(E1-d) ~ sinE1 - d*cosE1)
  dr/ddr via the orbital-plane -> inertial rotation, out = [dr | ddr]  [4,6]

Accuracy (vs the 2000-step damped-Newton f32 reference, worst case over
M in [0,1)): Newton-2+rotate ~6e-4, magic-rsqrt+1NR ~3.5e-3 -> ~4e-3
total, vs the 2e-2 gate.

Schedule: the serial Newton chain (3 Sin ACTs + ~21 tiny vector ops)
runs on Scalar+Vector; everything independent of E (rotation matrix C,
V = [3a, 3a(1-e^2), x^2+y^2], magic-rsqrt refine, u2 = -mm^2 a^3
rsqrt(xx+yy)*[x,y]) runs concurrently on GpSimd (tensor_tensor only --
Pool rejects TensorScalarPtr and 32-bit shifts, so constants arrive as
input lanes and the 2-op magic seed runs in a Vector ACT-wait gap).
The mm^2 a^3 product chain runs on the otherwise-idle Scalar engine via
Square/Copy activations. Host packs angles with -pi/2 offsets pre-added
so cos comes from the single Sin table (no Sqrt table).

Sharding: problem is tiny ("too small to shard") -> replicated SPMD on
all 8 cores; core 0's output is returned.
"""
import sys

if "/opt/trn_rl_repo" not in sys.path:
    sys.path.insert(0, "/opt/trn_rl_repo")

import numpy as np

N_ORBITS = 4
N_IN = 25
N_OUT = 6
HALF_PI = float(np.float32(np.pi / 2))
MAGIC = 0x5F3759DF

_cache = {}


def _build():
    import concourse.tile as tile
    from concourse import bacc, mybir

    AF = mybir.ActivationFunctionType
    ALU = mybir.AluOpType
    F32 = mybir.dt.float32
    I32 = mybir.dt.int32
    P = N_ORBITS

    nc = bacc.Bacc("TRN2", target_bir_lowering=False, debug=False)
    IN = nc.dram_tensor("inp", [P, N_IN], F32, kind="ExternalInput")
    OUT = nc.dram_tensor("out", [P, N_OUT], F32, kind="ExternalOutput")

    with tile.TileContext(nc) as tc:
        with tc.tile_pool(name="p", bufs=1) as pool:
            tin = pool.tile([P, N_IN], F32, tag="tin")
            nc.sync.dma_start(tin[:], IN.ap())

            m_ap = tin[:, 0:1]
            e_ap = tin[:, 11:12]
            a_ap = tin[:, 12:13]
            mm_ap = tin[:, 13:14]
            xy_ap = tin[:, 14:16]
            offs2 = tin[:, 16:18]    # [0, -pi/2]
            sgn_pm = tin[:, 18:20]   # [-1, +1]
            neg1 = tin[:, 18:19]
            c3 = tin[:, 20:21]       # 3.0
            nhalf = tin[:, 21:22]    # -0.5
            c1p5 = tin[:, 22:23]     # 1.5
            nxy = tin[:, 23:25]      # [-x, -y]

            # T = sin(angles): cols 0..10 =
            # [M, w-pi/2, w, W, W-pi/2, w, pi/2-w, i, i-pi/2, W-pi/2, W]
            # -> [sM, n_w, s_w, s_W, n_W, s_w, c_w, s_i, n_i, n_W, s_W]
            # (s_* = sin, n_* = -cos, c_* = +cos)
            T = pool.tile([P, 11], F32, tag="T")
            nc.scalar.activation(T[:], tin[:, 0:11], AF.Sin)
            sa1 = pool.tile([P, 1], F32, tag="sa1")  # mm^2
            nc.scalar.square(sa1[:], mm_ap)

            # ---- GpSimd side-channel (everything independent of E) ----
            # V = [3a, 3a(1-e^2), x^2+y^2, F0]; lane 3 is the Newton seed
            # F0 = e*sinM -- writing it here makes the magic-rsqrt shift
            # (which reads V[:, 0:4]) depend on F0, pinning the scheduler
            # so the critical-chain seed runs before the rsqrt side work.
            V = pool.tile([P, 4], F32, tag="V")
            nc.gpsimd.tensor_tensor(out=V[:, 0:1], in0=a_ap, in1=c3,
                                    op=ALU.mult)
            e2g = pool.tile([P, 1], F32, tag="e2g")
            nc.gpsimd.tensor_tensor(out=e2g[:], in0=e_ap, in1=e_ap,
                                    op=ALU.mult)
            tv = pool.tile([P, 1], F32, tag="tv")   # e^2 * 3a
            nc.gpsimd.tensor_tensor(out=tv[:], in0=e2g[:], in1=V[:, 0:1],
                                    op=ALU.mult)
            nc.gpsimd.tensor_tensor(out=V[:, 1:2], in0=V[:, 0:1], in1=tv[:],
                                    op=ALU.subtract)
            ne = pool.tile([P, 1], F32, tag="ne")   # -e
            nc.gpsimd.tensor_tensor(out=ne[:], in0=e_ap, in1=neg1,
                                    op=ALU.mult)
            nae = pool.tile([P, 1], F32, tag="nae")  # -a*e
            nc.gpsimd.tensor_tensor(out=nae[:], in0=ne[:], in1=a_ap,
                                    op=ALU.mult)
            # rotation-matrix pieces that only need T
            C = pool.tile([P, 6], F32, tag="C")  # [c11,c21,c31,c12,c22,c32]
            Cv = C[:].rearrange("p (h j) -> p h j", h=2)[:, :, 0:2]
            nc.gpsimd.tensor_tensor(out=C[:, 2:3], in0=T[:, 2:3],
                                    in1=T[:, 7:8], op=ALU.mult)  # c31=sw*si
            nw2 = pool.tile([P, 2], F32, tag="nw2")  # [cw, -sw]
            nc.gpsimd.tensor_tensor(out=nw2[:], in0=T[:, 1:3],
                                    in1=neg1.broadcast_to([P, 2]),
                                    op=ALU.mult)
            LWc = pool.tile([P, 2], F32, tag="LWc")  # [cW, sW]
            nc.gpsimd.tensor_tensor(out=LWc[:], in0=T[:, 9:11], in1=sgn_pm,
                                    op=ALU.mult)
            nc.gpsimd.tensor_tensor(out=C[:, 5:6], in0=nw2[:, 0:1],
                                    in1=T[:, 7:8], op=ALU.mult)  # c32=cw*si
            # (magic-rsqrt refine emitted after the Vector seed ops below --
            # tile deps are tracked in emission order, so a pool read of Y
            # emitted before Y's write would silently get no dependency)
            Y = pool.tile([P, 4], F32, tag="Y")

            # ---- Vector: idle-start fillers, then the Newton chain ----
            sq2 = pool.tile([P, 2], F32, tag="sq2")
            nc.vector.tensor_tensor(out=sq2[:], in0=xy_ap, in1=xy_ap,
                                    op=ALU.mult)
            nc.vector.tensor_tensor(out=V[:, 2:3], in0=sq2[:, 0:1],
                                    in1=sq2[:, 1:2], op=ALU.add)

            # seed: F0 = e*sinM -> V[:, 3]; EE0 = [F0, F0 - pi/2]
            F0 = V[:, 3:4]
            nc.vector.tensor_scalar(out=F0, in0=T[:, 0:1], scalar1=e_ap,
                                    scalar2=None, op0=ALU.mult)
            EE0 = pool.tile([P, 2], F32, tag="EE0")
            nc.vector.tensor_tensor(out=EE0[:], in0=F0.broadcast_to([P, 2]),
                                    in1=offs2, op=ALU.add)
            S1 = pool.tile([P, 2], F32, tag="S1")
            nc.scalar.activation(S1[:], EE0[:], AF.Sin, bias=m_ap)
            sa3 = pool.tile([P, 1], F32, tag="sa3")  # mm^2 a
            nc.scalar.mul(sa3[:], sa1[:], a_ap)
            sa2 = pool.tile([P, 1], F32, tag="sa2")  # a^2
            nc.scalar.square(sa2[:], a_ap)

            # gap fillers during S1: magic-rsqrt seed (shift needs Vector)
            sh = pool.tile([P, 4], I32, tag="sh")
            nc.vector.tensor_scalar(out=sh[:], in0=V[:].bitcast(I32),
                                    scalar1=1, scalar2=None,
                                    op0=ALU.logical_shift_right)
            nc.vector.tensor_scalar(out=Y[:].bitcast(I32), in0=sh[:],
                                    scalar1=MAGIC, scalar2=-1,
                                    op0=ALU.subtract, op1=ALU.mult)

            # GpSimd: magic-rsqrt Newton-Raphson refine + SQpm
            nr = pool.tile([P, 4], F32, tag="nr")
            nc.gpsimd.tensor_tensor(out=nr[:], in0=Y[:], in1=Y[:],
                                    op=ALU.mult)
            nc.gpsimd.tensor_tensor(out=nr[:], in0=nr[:], in1=V[:],
                                    op=ALU.mult)
            nc.gpsimd.tensor_tensor(out=nr[:], in0=nr[:],
                                    in1=nhalf.broadcast_to([P, 4]),
                                    op=ALU.mult)
            nc.gpsimd.tensor_tensor(out=nr[:], in0=nr[:],
                                    in1=c1p5.broadcast_to([P, 4]),
                                    op=ALU.add)
            Y1 = pool.tile([P, 4], F32, tag="Y1")
            nc.gpsimd.tensor_tensor(out=Y1[:], in0=Y[:], in1=nr[:],
                                    op=ALU.mult)
            # SQpm = [-sqrt(3a), +sqrt(3a(1-e^2))]
            SQ = pool.tile([P, 2], F32, tag="SQ")
            nc.gpsimd.tensor_tensor(out=SQ[:], in0=V[:, 0:2], in1=Y1[:, 0:2],
                                    op=ALU.mult)
            SQpm = pool.tile([P, 2], F32, tag="SQpm")
            nc.gpsimd.tensor_tensor(out=SQpm[:], in0=SQ[:], in1=sgn_pm,
                                    op=ALU.mult)

            # Newton iteration 1
            f1 = pool.tile([P, 1], F32, tag="f1")
            nc.vector.tensor_scalar(out=f1[:], in0=S1[:, 0:1], scalar1=ne[:],
                                    scalar2=F0, op0=ALU.mult, op1=ALU.add)
            d1 = pool.tile([P, 1], F32, tag="d1")
            nc.vector.tensor_scalar(out=d1[:], in0=S1[:, 1:2], scalar1=e_ap,
                                    scalar2=1.0, op0=ALU.mult, op1=ALU.add)
            r1 = pool.tile([P, 1], F32, tag="r1")
            nc.vector.reciprocal(r1[:], d1[:])
            dF1 = pool.tile([P, 1], F32, tag="dF1")
            nc.vector.tensor_tensor(out=dF1[:], in0=f1[:], in1=r1[:],
                                    op=ALU.mult)
            EE1 = pool.tile([P, 2], F32, tag="EE1")
            nc.vector.tensor_tensor(out=EE1[:], in0=EE0[:],
                                    in1=dF1[:].broadcast_to([P, 2]),
                                    op=ALU.subtract)
            S2 = pool.tile([P, 2], F32, tag="S2")
            nc.scalar.activation(S2[:], EE1[:], AF.Sin, bias=m_ap)
            sa4 = pool.tile([P, 1], F32, tag="sa4")  # mm^2 a^3
            nc.scalar.mul(sa4[:], sa3[:], sa2[:])
            # w2 = -mm^2 a^3 * [x, y]  (sign via -x,-y lanes); the
            # remaining rsqrt(xx+yy)*rci^2 factor lands in the tail.
            w2 = pool.tile([P, 2], F32, tag="w2")
            nc.gpsimd.tensor_tensor(out=w2[:], in0=nxy,
                                    in1=sa4[:].broadcast_to([P, 2]),
                                    op=ALU.mult)

            # gap fillers during S2: rotation-matrix outer products
            C4m = pool.tile([P, 2, 2], F32, tag="C4m")
            nc.vector.tensor_tensor(
                out=C4m[:], in0=nw2[:].unsqueeze(2).broadcast_to([P, 2, 2]),
                in1=LWc[:].unsqueeze(1).broadcast_to([P, 2, 2]), op=ALU.mult)
            z2 = pool.tile([P, 2], F32, tag="z2")  # [-sw*ci, -cw*ci]
            nc.vector.tensor_scalar(out=z2[:], in0=T[:, 5:7],
                                    scalar1=T[:, 8:9], scalar2=None,
                                    op0=ALU.mult)

            # GpSimd finishes C once z2/C4m land
            Cb4 = pool.tile([P, 2, 2], F32, tag="Cb4")
            nc.gpsimd.tensor_tensor(
                out=Cb4[:], in0=z2[:].unsqueeze(2).broadcast_to([P, 2, 2]),
                in1=T[:, 3:5].unsqueeze(1).broadcast_to([P, 2, 2]),
                op=ALU.mult)
            nc.gpsimd.tensor_tensor(out=Cv, in0=C4m[:], in1=Cb4[:],
                                    op=ALU.add)

            # Newton iteration 2 + first-order trig rotation
            f2 = pool.tile([P, 1], F32, tag="f2")
            nc.vector.tensor_scalar(out=f2[:], in0=S2[:, 0:1], scalar1=ne[:],
                                    scalar2=EE1[:, 0:1], op0=ALU.mult,
                                    op1=ALU.add)
            d2 = pool.tile([P, 1], F32, tag="d2")
            nc.vector.tensor_scalar(out=d2[:], in0=S2[:, 1:2], scalar1=e_ap,
                                    scalar2=1.0, op0=ALU.mult, op1=ALU.add)
            r2 = pool.tile([P, 1], F32, tag="r2")
            nc.vector.reciprocal(r2[:], d2[:])
            dF2 = pool.tile([P, 1], F32, tag="dF2")
            nc.vector.tensor_tensor(out=dF2[:], in0=f2[:], in1=r2[:],
                                    op=ALU.mult)
            # S5 = [sin(Ef), cos(Ef)], Ef = E1 - dF2:
            #   sin ~ S2_0 + dF2*S2_1 ; cos ~ dF2*S2_0 - S2_1
            S5 = pool.tile([P, 2], F32, tag="S5")
            nc.vector.tensor_scalar(out=S5[:, 0:1], in0=S2[:, 1:2],
                                    scalar1=dF2[:], scalar2=S2[:, 0:1],
                                    op0=ALU.mult, op1=ALU.add)
            nc.vector.tensor_scalar(out=S5[:, 1:2], in0=S2[:, 0:1],
                                    scalar1=dF2[:], scalar2=S2[:, 1:2],
                                    op0=ALU.mult, op1=ALU.subtract)

            # ---- tail ----
            rcen = pool.tile([P, 1], F32, tag="rcen")  # a(1 - e cosEf)
            nc.vector.tensor_scalar(out=rcen[:], in0=S5[:, 1:2],
                                    scalar1=nae[:], scalar2=a_ap,
                                    op0=ALU.mult, op1=ALU.add)
            rci = pool.tile([P, 1], F32, tag="rci")
            nc.vector.reciprocal(rci[:], rcen[:])
            sc2 = pool.tile([P, 2], F32, tag="sc2")
            nc.vector.tensor_scalar(out=sc2[:], in0=SQpm[:], scalar1=rci[:],
                                    scalar2=None, op0=ALU.mult)
            PQ = pool.tile([P, 4], F32, tag="PQ")  # [dx, px, dy, qx]
            nc.vector.tensor_tensor(out=PQ[:, 0:4:2], in0=sc2[:], in1=S5[:],
                                    op=ALU.mult)
            q = pool.tile([P, 1], F32, tag="q")
            nc.vector.tensor_scalar(out=q[:], in0=Y1[:, 2:3],
                                    scalar1=rci[:], scalar2=rci[:],
                                    op0=ALU.mult, op1=ALU.mult)
            nc.vector.tensor_scalar(out=PQ[:, 1:4:2], in0=w2[:],
                                    scalar1=q[:], scalar2=None,
                                    op0=ALU.mult)

            O1 = pool.tile([P, 6], F32, tag="O1")
            nc.vector.tensor_tensor(
                out=O1[:].rearrange("p (h j) -> p h j", h=2),
                in0=C[:, 0:3].unsqueeze(1).broadcast_to([P, 2, 3]),
                in1=PQ[:, 0:2].unsqueeze(2).broadcast_to([P, 2, 3]),
                op=ALU.mult)
            O2 = pool.tile([P, 6], F32, tag="O2")
            nc.vector.tensor_tensor(
                out=O2[:].rearrange("p (h j) -> p h j", h=2),
                in0=C[:, 3:6].unsqueeze(1).broadcast_to([P, 2, 3]),
                in1=PQ[:, 2:4].unsqueeze(2).broadcast_to([P, 2, 3]),
                op=ALU.mult)
            Ot = pool.tile([P, 6], F32, tag="Ot")
            nc.vector.tensor_tensor(out=Ot[:], in0=O1[:], in1=O2[:],
                                    op=ALU.add)
            nc.sync.dma_start(OUT.ap(), Ot[:])

    nc.compile()
    return nc


def _pack(a, e, i, omega, Omega, mean_motion, mean_anomaly, x):
    P = N_ORBITS
    IN = np.zeros((P, N_IN), np.float32)
    M = np.full((P,), np.float32(mean_anomaly), np.float32)
    w = np.asarray(omega, np.float32).reshape(P)
    W = np.asarray(Omega, np.float32).reshape(P)
    ii = np.asarray(i, np.float32).reshape(P)
    xf = np.asarray(x, np.float32)
    IN[:, 0] = M
    IN[:, 1] = w - HALF_PI
    IN[:, 2] = w
    IN[:, 3] = W
    IN[:, 4] = W - HALF_PI
    IN[:, 5] = w
    IN[:, 6] = HALF_PI - w
    IN[:, 7] = ii
    IN[:, 8] = ii - HALF_PI
    IN[:, 9] = W - HALF_PI
    IN[:, 10] = W
    IN[:, 11] = np.asarray(e, np.float32).reshape(P)
    IN[:, 12] = np.asarray(a, np.float32).reshape(P)
    IN[:, 13] = np.asarray(mean_motion, np.float32).reshape(P)
    IN[:, 14] = xf[:, 0]
    IN[:, 15] = xf[:, 1]
    IN[:, 16] = 0.0
    IN[:, 17] = -HALF_PI
    IN[:, 18] = -1.0
    IN[:, 19] = 1.0
    IN[:, 20] = 3.0
    IN[:, 21] = -0.5
    IN[:, 22] = 1.5
    IN[:, 23] = -xf[:, 0]
    IN[:, 24] = -xf[:, 1]
    return IN


def kernel(a, e, i, omega, Omega, mean_motion, mean_anomaly, x, _trace=False):
    from concourse.bass_utils import run_bass_kernel_spmd

    if "nc" not in _cache:
        _cache["nc"] = _build()
    nc = _cache["nc"]

    IN = _pack(a, e, i, omega, Omega, mean_motion, mean_anomaly, x)
    n_cores = 1 if _trace else 8
    res = run_bass_kernel_spmd(nc, [{"inp": IN}] * n_cores,
                               core_ids=list(range(n_cores)), trace=_trace)
    out = res.results[0]["out"].astype(np.float32)
    if _trace:
        _cache["last_result"] = res
    return out
